# revision 53
# baseline (speedup 1.0000x reference)
"""Additive (Bahdanau) attention for Trainium2, SPMD over 8 NeuronCores.

Reference (per batch b):
    e[i,k] = sum_d tanh(q[i,d] + v[k,d]);  w = softmax_k(e);  out = w @ v
Shapes: B=4, Tq=Tk=512, D=128, fp32. 8 shards = (batch, half of Tq); each
core computes a [256,128] output slice independently (no collectives).

Algorithm: tanh(a+b) ~= sum_r alpha_r sin(w_r(a+b)) =
sum_r alpha_r [sin(w_r a)cos(w_r b) + cos(w_r a)sin(w_r b)], so the logits
become a matmul over d between per-element Fourier features of q and v --
replacing the baseline's O(Tq*Tk*D) elementwise tanh (ACT-bound, ~114us)
with O((Tq+Tk)*D) features + fp16 PE matmuls. Frequencies: two seeds
(0.3128, 0.44), each a depth-3 doubling ladder (8 freqs, 16 rank-1 terms).
Weighted LS fit vs tanh under the N(0,2) input measure; end-to-end error
6.4e-3 (gate 2e-2) including fp16 feature rounding and bf16 output path.

ACT's Sin spline is only valid to |arg|<~3.9 (no range reduction), so only
seed frequencies are computed on ACT (args <= 0.44*5 + pi/2 < 3.8); higher
frequencies are derived on DVE with exact fp16 product recursions (scale
factors tracked and folded into the per-pair alpha scaling):
    q side (affine-clean -- additive constants here would create v-only
    logit terms that softmax does NOT cancel):
        P_l = P_{l-1}*C_{l-1}                      (= sin(2^l th)/2^l)
        C_l = (C_{l-1}+2^{l-1}P_{l-1})(C_{l-1}-2^{l-1}P_{l-1})  (= cos)
    v side (constants allowed -- they become q-only terms, which softmax
    kills): P/Q/C with  C_l = 1 - 2*4^{l-1} Q_l, Q from ACT Square where
    it is off the critical recursion. Terminal levels expand cos through Q
    (the q-only residue dies in softmax) and pre-fold the pair alphas into
    the producing scalar_tensor_tensor ops.

Logits accumulate transposed (eT chunks [k=128p, i=256], one PSUM bank
each -- one pending accumulation group per 2KB bank is a hard limit), so
softmax needs no transposes: Exp output w^T chunks (bf16; exp fits fp32/
bf16 range without max-subtraction, |e|<~36) feed the output matmul as
lhsT against bf16 V augmented with a ones column (out col 128 = softmax
denominator; DVE reciprocal + ACT scale normalize).

Engine schedule (all sem-counted). GPSIMD only holds work whose results
are consumed much later (constants, the bf16 V copy): on device its
then_inc can fire before the Q7's SBUF writes are visible, so tightly
consumed gp produce-consume chains (e.g. alpha-scales) return garbage --
verified empirically; keep such scaling on DVE/ACT. All consumers of gp
constants wait for the FULL s_const count (per-Q7 completion reorders).
  sync : q DMA (rearranged [128,2,128]), v chunks 0-1, output o0
  ACT  : v chunks 2-3 DMA, 8 base Sin (free-affine scale/bias), 4 Square,
         2 late alpha-scales, 4 Exp, 2 normalize muls, output o1
  DVE  : 1 square + 36 ladder ops + 10 alpha-scales + 2 reciprocals
  GPSIMD: constants + bf16 copy of V (counting-safe: consumers wait for
         the full group)
  PE   : 6 fp32 transposes, 64 fp16 feature MMs (N=256), 8 bf16 output
         MMs (N=129)
"""

from contextlib import ExitStack

import numpy as np

B, TQ_FULL, TK, D = 4, 512, 512, 128
N_CORES = 8
TQ = TQ_FULL * B // N_CORES  # 256 q-rows per core
KT = TK // 128               # 4 k-chunks

SEEDS = (0.3128300658904047, 0.44)
ALPHAS = (
    (1.9340600433649018, 0.6047390403046087, 0.1333800440130172,
     0.03431347670397111),
    (-1.1005360250056995, 0.08256326460903496, 0.0764390461692493,
     0.009518297546871847),
)
PI2 = float(np.pi / 2)

_NC_CACHE = {}


def _build_nc():
    import concourse.bass as bass
    import concourse.mybir as mybir

    f32 = mybir.dt.float32
    f16 = mybir.dt.float16
    bf16 = mybir.dt.bfloat16
    AF = mybir.ActivationFunctionType
    OP = mybir.AluOpType

    nc = bass.Bass(trn_type="TRN2")
    q_d = nc.dram_tensor("query", (TQ, D), f32, kind="ExternalInput")
    v_d = nc.dram_tensor("value", (TK, D), f32, kind="ExternalInput")
    o_d = nc.dram_tensor("out", (TQ, D), f32, kind="ExternalOutput")

    ctx = ExitStack()
    with ctx:
        sb = lambda name, shape, dt: ctx.enter_context(
            nc.sbuf_tensor(name, shape, dt)
        )
        ps = lambda name, shape: ctx.enter_context(
            nc.psum_tensor(name, shape, f32)
        )
        sem = lambda name: ctx.enter_context(nc.semaphore(name))

        ident = sb("ident", [128, 128], f32)
        b_pi2 = sb("b_pi2", [128, 1], f32)
        q_nat = sb("q_nat", [128, 2, D], f32)
        v_nat = sb("v_nat", [128, KT, D + 1], f32)

        sq = [sb(f"sq{s}", [128, 256], f16) for s in range(2)]
        cq = [sb(f"cq{s}", [128, 256], f16) for s in range(2)]
        sv = [sb(f"sv{s}", [128, 512], f16) for s in range(2)]
        cv = [sb(f"cv{s}", [128, 512], f16) for s in range(2)]
        qP = [[None] + [sb(f"qP{s}{l}", [128, 256], f16) for l in (1, 2, 3)]
              for s in range(2)]
        qA = [[None] + [sb(f"qA{s}{l}", [128, 256], f16) for l in (1, 2, 3)]
              for s in range(2)]
        qB = [[None] + [sb(f"qB{s}{l}", [128, 256], f16) for l in (1, 2, 3)]
              for s in range(2)]
        qC = [[None] + [sb(f"qC{s}{l}", [128, 256], f16) for l in (1, 2, 3)]
              for s in range(2)]
        qM = [[None] + [sb(f"qM{s}{l}", [128, 256], f16) for l in (1, 2, 3)]
              for s in range(2)]
        vP = [[None] + [sb(f"vP{s}{l}", [128, 512], f16) for l in (1, 2, 3)]
              for s in range(2)]
        vQ = [[None] + [sb(f"vQ{s}{l}", [128, 512], f16) for l in (1, 2, 3)]
              for s in range(2)]
        vC = [[None] + [sb(f"vC{s}{l}", [128, 512], f16) for l in (1, 2)]
              for s in range(2)]
        aq = [sb(f"aq{p}", [128, 256], f16) for p in range(16)]

        w_sb = sb("w_sb", [128, KT, 256], bf16)
        v16 = sb("v16", [128, KT, D + 1], bf16)
        rs = [sb(f"rs{it}", [128, 1], f32) for it in range(2)]
        o_sb = sb("o_sb", [128, 2, D], f32)

        # PSUM: 7 of 8 banks.
        bq = ps("bq", [128, 512])      # qT in cols 0:256
        bv = ps("bv", [128, 512])      # vT
        e_ps = [ps(f"e{kt}", [128, 512]) for kt in range(KT)]
        bo = [ps(f"bo{it}", [128, 512]) for it in range(2)]
        warm = bq  # fillers run before the first transpose? no - use bo[1]
        # fillers write bo[1][:, 256:384]; the out group (cols 0:129) starts
        # with start=True and never reads that region.

        s_dmaq = sem("s_dmaq")    # q tile 0 (gp SWDGE)
        s_dmaq2 = sem("s_dmaq2")  # q tile 1 (sync)
        s_dmav = sem("s_dmav")    # v chunks 0-1 (sync)
        s_dmav2 = sem("s_dmav2")  # v chunks 2-3 (DVE-dispatched)
        s_const = sem("s_const")  # gpsimd consts
        s_tp = sem("s_tp")        # PE transposes (6)
        s_feat = sem("s_feat")    # ACT base features (8)
        s_sq = sem("s_sq")        # ACT terminal squares (2)
        s_dve = sem("s_dve")      # DVE derivation ops
        s_alpha = sem("s_alpha")  # gpsimd alpha-scales (12)
        s_alpha2 = sem("s_alpha2")  # DVE inline alpha-scales (4)
        s_mm = sem("s_mm")        # PE last-pair MM per chunk (4)
        s_w = sem("s_w")          # ACT exp per chunk (4)
        s_o = sem("s_o")          # PE out-MM group per i-tile (2)
        s_rs = sem("s_rs")        # DVE reciprocal fence
        s_norm = sem("s_norm")    # DVE normalized outputs (2)
        s_outd = sem("s_outd")    # output DMAs
        s_v16 = sem("s_v16")      # bf16 copy of v_nat ready

        # ---- DVE derivation stream ---------------------------------------
        # op kinds: tt(out,a,b,alu) ts(out,a,s1,s2)
        # a2(out,src,scale) -> DVE-inline alpha, incs s_alpha2
        # ag(slot,src,scale) -> extracted to gpsimd, incs s_alpha by 2^k
        def L_q(s, l):
            if l == 1:
                return [
                    ("tt", qP[s][1], sq[s], cq[s], "mult"),
                    ("tt", qA[s][1], cq[s], sq[s], "add"),
                    ("tt", qB[s][1], cq[s], sq[s], "subtract"),
                    ("tt", qC[s][1], qA[s][1], qB[s][1], "mult"),
                ]
            m = float(2 ** (l - 1))
            return [
                ("tt", qP[s][l], qP[s][l - 1], qC[s][l - 1], "mult"),
                ("ts", qM[s][l], qP[s][l - 1], m, 0.0),
                ("tt", qA[s][l], qC[s][l - 1], qM[s][l], "add"),
                ("tt", qB[s][l], qC[s][l - 1], qM[s][l], "subtract"),
                ("tt", qC[s][l], qA[s][l], qB[s][l], "mult"),
            ]

        def L_v(s, l):
            # vQ tiles other than (0,1) are produced by ACT Square.
            if l == 1 and s == 0:
                return [
                    ("tt", vP[0][1], sv[0], cv[0], "mult"),
                    ("tt", vQ[0][1], sv[0], sv[0], "mult"),
                    ("ts", vC[0][1], vQ[0][1], -2.0, 1.0),
                ]
            if l == 1:
                return [
                    ("tt", vP[1][1], sv[1], cv[1], "mult"),
                    ("ts", vC[1][1], vQ[1][1], -2.0, 1.0),
                ]
            if l == 2:
                return [
                    ("tt", vP[s][2], vP[s][1], vC[s][1], "mult"),
                    ("ts", vC[s][2], vQ[s][2], -8.0, 1.0),
                ]
            return [("stt", vP[s][3], vP[s][2], A3[s][1], vC[s][2], "mult")]

        A3 = [(-256.0 * ALPHAS[s][3], 8.0 * ALPHAS[s][3]) for s in range(2)]

        def q3_block(s):
            # qP[s][3] is emitted pre-scaled by A3[s][0] (fold into the
            # product via STT); the cos-chain rescales it back via qM.
            cp = A3[s][0]
            return [
                ("stt", qP[s][3], qP[s][2], cp, qC[s][2], "mult"),
                ("ts", qM[s][3], qP[s][2], 4.0, 0.0),
                ("tt", qA[s][3], qC[s][2], qM[s][3], "add"),
                ("tt", qB[s][3], qC[s][2], qM[s][3], "subtract"),
                ("tt", qC[s][3], qA[s][3], qB[s][3], "mult"),
            ]

        # gp alpha helper: slot fixed per pair below
        def ag(slot, src, scale):
            return ("ag", slot, src, float(scale))

        # DVE stream: the q-side ladder of seed 0 first (depends only on
        # sq0/cq0), then v-sides and seed 1 as features land. The ag entries
        # mark where gpsimd alpha-scales become runnable; their order here
        # is gpsimd's emission order.
        dve_ops = (
            [ag(0, sq[0], ALPHAS[0][0]), ag(1, cq[0], ALPHAS[0][0])]
            + L_q(0, 1)
            + [ag(4, qP[0][1], 2 * ALPHAS[0][1]),
               ag(5, qC[0][1], 2 * ALPHAS[0][1])]
            + L_q(0, 2)
            + [ag(6, qP[0][2], 4 * ALPHAS[0][2]),
               ag(7, qC[0][2], 4 * ALPHAS[0][2])]
            + q3_block(0)
            + L_v(0, 1)
            + [ag(2, sq[1], ALPHAS[1][0]), ag(3, cq[1], ALPHAS[1][0])]
            + L_q(1, 1)
            + [ag(8, qP[1][1], 2 * ALPHAS[1][1]),
               ag(9, qC[1][1], 2 * ALPHAS[1][1])]
            + L_v(0, 2) + L_v(0, 3) + L_v(1, 1)
            + L_q(1, 2)
            + [ag(10, qP[1][2], 4 * ALPHAS[1][2]),
               ag(11, qC[1][2], 4 * ALPHAS[1][2])]
            + L_v(1, 2) + L_v(1, 3)
            + q3_block(1)
        )

        # ag slots 0-7 stay on DVE (needed early, in-order engine);
        # slots 8-11 run on ACT's idle window (strict FIFO, counting-safe),
        # interleaved with the squares by dependency.
        ACT_AG = [op for op in dve_ops if op[0] == "ag" and op[1] >= 8]
        dve_ops = [op for op in dve_ops
                   if not (op[0] == "ag" and op[1] >= 8)]
        DVE_AG = [op for op in dve_ops if op[0] == "ag"]

        DVE_IDX, n_dve = {}, 0
        for op in dve_ops:
            if op[0] not in ("a2", "ag"):
                n_dve += 1
                DVE_IDX[id(op[1])] = n_dve

        FEAT_IDX = {id(sq[0]): 1, id(cq[0]): 2, id(sv[0]): 3, id(cv[0]): 4,
                    id(sq[1]): 5, id(cq[1]): 6, id(sv[1]): 7, id(cv[1]): 8}

        # ACT-square schedule: (out_tile, in_tile). Emitted in this order;
        # s_sq counts completions. Deps: first is feature-only, rest on DVE.
        SQ_OPS = [
            (vQ[1][1], sv[1]),
            (vQ[0][2], vP[0][1]),
            (vQ[0][3], vP[0][2]),
            (vQ[1][2], vP[1][1]),
            (vQ[1][3], vP[1][2]),
        ]
        SQ_IDX = {}  # populated by the ACT-mid merge (build_act_mid)

        # ACT mid-section: squares + late alphas, merged by DVE dependency.
        ACT_MID = []
        for out_t, in_t in SQ_OPS:
            dep = ((FEAT_IDX[id(in_t)], 0) if id(in_t) in FEAT_IDX
                   else (0, DVE_IDX[id(in_t)]))
            ACT_MID.append((dep, "sq", out_t, in_t, None))
        for _kind, slot, src_, scl in ACT_AG:
            ACT_MID.append(((0, DVE_IDX[id(src_)]), "ag", slot, src_, scl))
        ACT_MID.sort(key=lambda m: (m[0][1], m[0][0]))
        nsq = 0
        AACT_IDX = {}
        for m in ACT_MID:
            if m[1] == "sq":
                nsq += 1
                SQ_IDX[id(m[2])] = nsq
            else:
                AACT_IDX[m[2]] = len(AACT_IDX) + 1


        # DVE waits on ACT features / ACT squares, from each op's inputs.
        dve_waits = {}
        seenf = seens = 0
        for i, op in enumerate(dve_ops):
            ins = ([op[2], op[3]] if op[0] == "tt"
                   else [op[2], op[4]] if op[0] == "stt" else [op[2]])
            needf = max(
                (FEAT_IDX[id(t)] for t in ins if id(t) in FEAT_IDX),
                default=0,
            )
            needs = max(
                (SQ_IDX[id(t)] for t in ins if id(t) in SQ_IDX),
                default=0,
            )
            w = []
            if needf > seenf:
                w.append(("feat", needf))
                seenf = needf
            if needs > seens:
                w.append(("sq", needs))
                seens = needs
            if w:
                dve_waits[i] = w

        # ---- MM pairs, ordered by operand availability -------------------
        # (q_tile, aq_slot, v_tile, v_dep, alpha_dep)
        # alpha_dep: ("g", wait_value) on s_alpha | ("d", n) on s_alpha2
        _dve_ag_order = [op[1] for op in DVE_AG]
        _act_ag_order = [op[1] for op in ACT_AG]

        def gdep(slot):
            if slot in _dve_ag_order:
                return ("g", (1 << (_dve_ag_order.index(slot) + 1)) - 1)
            return ("a", AACT_IDX[slot])

        def dv(t):
            return ("dve", DVE_IDX[id(t)])

        pairs = [
            (0, cv[0], ("feat", 4), gdep(0)),
            (1, sv[0], ("feat", 3), gdep(1)),
            (5, vP[0][1], dv(vP[0][1]), gdep(5)),
            (4, vC[0][1], dv(vC[0][1]), gdep(4)),
            (3, sv[1], ("feat", 7), gdep(3)),
            (2, cv[1], ("feat", 8), gdep(2)),
            (7, vP[0][2], dv(vP[0][2]), gdep(7)),
            (6, vC[0][2], dv(vC[0][2]), gdep(6)),
            (-1, vP[0][3], dv(vP[0][3]),
             ("raw", qC[0][3], dv(qC[0][3]))),
            (-1, vQ[0][3], ("sq", 3),
             ("raw", qP[0][3], dv(qP[0][3]))),
            (9, vP[1][1], dv(vP[1][1]), gdep(9)),
            (8, vC[1][1], dv(vC[1][1]), gdep(8)),
            (11, vP[1][2], dv(vP[1][2]), gdep(11)),
            (10, vC[1][2], dv(vC[1][2]), gdep(10)),
            (-1, vQ[1][3], ("sq", 5), ("raw", qP[1][3], dv(qP[1][3]))),
            (-1, vP[1][3], dv(vP[1][3]), ("raw", qC[1][3], dv(qC[1][3]))),
        ]
        assert len(pairs) == 16

        with nc.Block() as block:

            @block.gpsimd
            def _(gp):
                nc.gpsimd.memset(ident[:, :], 0.0).then_inc(s_const, 1)
                gp.wait_ge(s_const, 1)
                nc.gpsimd.affine_select(
                    out=ident[:, :],
                    in_=ident[:, :],
                    compare_op=mybir.AluOpType.not_equal,
                    fill=1.0,
                    base=0,
                    pattern=[[-1, 128]],
                    channel_multiplier=1,
                ).then_inc(s_const, 1)
                nc.gpsimd.memset(b_pi2[:, :], PI2).then_inc(s_const, 1)
                nc.gpsimd.memset(v_nat[:, :, D : D + 1], 1.0).then_inc(
                    s_const, 1
                )
                nc.gpsimd.memset(v16[:, :, D : D + 1], 1.0).then_inc(
                    s_const, 1
                )
                gp.wait_ge(s_dmav, 16)
                nc.gpsimd.tensor_copy(
                    v16[:, 0:2, 0:D], v_nat[:, 0:2, 0:D]
                ).then_inc(s_v16, 1)
                gp.wait_ge(s_dmav2, 16)
                nc.gpsimd.tensor_copy(
                    v16[:, 2:4, 0:D], v_nat[:, 2:4, 0:D]
                ).then_inc(s_v16, 1)

                # Alpha-scales for the gpsimd-owned pairs.
                fdone = ddone = 0
                for p in GP_PAIRS:
                    q_src, scale = pairs[p][0], pairs[p][1]
                    if id(q_src) in FEAT_IDX:
                        n = FEAT_IDX[id(q_src)]
                        if fdone < n:
                            gp.wait_ge(s_feat, n)
                            fdone = n
                    else:
                        n = DVE_IDX[id(q_src)]
                        if ddone < n:
                            gp.wait_ge(s_dve, n)
                            ddone = n
                    nc.gpsimd.tensor_scalar_mul(
                        aq[AQ_SLOT[p]][:, :], q_src[:, :], float(scale)
                    ).then_inc(s_alpha, 1)

            @block.sync
            def _(sp):
                v_re = v_d[:, :].rearrange("(kt kp) d -> kp kt d", kp=128)
                q_re = q_d[:, :].rearrange("(it ip) d -> ip it d", ip=128)
                sp.dma_start(out=q_nat[:, :, :], in_=q_re[:, :, :]).then_inc(
                    s_dmaq, 16
                )
                sp.dma_start(
                    out=v_nat[:, 0:2, 0:D], in_=v_re[:, 0:2, :]
                ).then_inc(s_dmav, 16)
                sp.wait_ge(s_norm, 1)
                sp.dma_start(out=o_d[0:128, :], in_=o_sb[:, 0, :]).then_inc(
                    s_outd, 16
                )
                sp.wait_ge(s_outd, 32)

            @block.tensor
            def _(pe):
                pe.wait_ge(s_const, 5)
                # Transposes in DMA-arrival order: q0, q1, v2, v3, v0, v1.
                pe.wait_ge(s_dmaq, 16)
                nc.tensor.transpose(
                    bq[:, 0:128], q_nat[:, 0, :], ident[:, :]
                ).then_inc(s_tp, 1)
                nc.tensor.transpose(
                    bq[:, 128:256], q_nat[:, 1, :], ident[:, :]
                ).then_inc(s_tp, 1)
                pe.wait_ge(s_dmav, 16)
                for kt in (0, 1):
                    nc.tensor.transpose(
                        bv[:, kt * 128 : (kt + 1) * 128],
                        v_nat[:, kt, 0:D],
                        ident[:, :],
                    ).then_inc(s_tp, 1)
                pe.wait_ge(s_dmav2, 16)
                for kt in (2, 3):
                    nc.tensor.transpose(
                        bv[:, kt * 128 : (kt + 1) * 128],
                        v_nat[:, kt, 0:D],
                        ident[:, :],
                    ).then_inc(s_tp, 1)

                fdone = ddone = adone = a2done = sqdone = 0
                for p, (slot, vt, (vk, vn), adep) in enumerate(pairs):
                    if vk == "feat" and fdone < vn:
                        pe.wait_ge(s_feat, vn)
                        fdone = vn
                    elif vk == "dve" and ddone < vn:
                        pe.wait_ge(s_dve, vn)
                        ddone = vn
                    elif vk == "sq" and sqdone < vn:
                        pe.wait_ge(s_sq, vn)
                        sqdone = vn
                    if adep[0] == "g":
                        if adone < adep[1]:
                            pe.wait_ge(s_alpha, adep[1])
                            adone = adep[1]
                    elif adep[0] == "d":
                        if a2done < adep[1]:
                            pe.wait_ge(s_alpha2, adep[1])
                            a2done = adep[1]
                    else:  # raw q tile straight from DVE
                        q_tile, (_k, qn) = adep[1], adep[2]
                        if ddone < qn:
                            pe.wait_ge(s_dve, qn)
                            ddone = qn
                    rhs = aq[slot] if slot >= 0 else adep[1]
                    for kt in range(KT):
                        mm = nc.tensor.matmul(
                            e_slice(kt),
                            vt[:, kt * 128 : (kt + 1) * 128],
                            rhs[:, :],
                            start=(p == 0),
                            stop=(p == 15),
                        )
                        if p == 15:
                            mm.then_inc(s_mm, 1)

                pe.wait_ge(s_v16, 2)
                for kt in range(KT):
                    pe.wait_ge(s_w, kt + 1)
                    for it in range(2):
                        mm = nc.tensor.matmul(
                            bo[it][:, 0 : D + 1],
                            w_sb[:, kt, it * 128 : (it + 1) * 128],
                            v16[:, kt, :],
                            start=(kt == 0),
                            stop=(kt == KT - 1),
                        )
                        if kt == KT - 1:
                            mm.then_inc(s_o, 1)

            @block.scalar
            def _(act):
                v_re2 = v_d[:, :].rearrange("(kt kp) d -> kp kt d", kp=128)
                nc.scalar.dma_start(
                    out=v_nat[:, 2:4, 0:D], in_=v_re2[:, 2:4, :]
                ).then_inc(s_dmav2, 16)
                act.wait_ge(s_const, 5)
                act.wait_ge(s_tp, 2)
                nc.scalar.activation(
                    sq[0][:, :], bq[:, 0:256], AF.Sin, scale=SEEDS[0]
                ).then_inc(s_feat, 1)
                nc.scalar.activation(
                    cq[0][:, :], bq[:, 0:256], AF.Sin,
                    bias=b_pi2[:, :], scale=SEEDS[0],
                ).then_inc(s_feat, 1)
                act.wait_ge(s_tp, 6)
                nc.scalar.activation(
                    sv[0][:, :], bv[:, :], AF.Sin, scale=SEEDS[0]
                ).then_inc(s_feat, 1)
                nc.scalar.activation(
                    cv[0][:, :], bv[:, :], AF.Sin,
                    bias=b_pi2[:, :], scale=SEEDS[0],
                ).then_inc(s_feat, 1)
                nc.scalar.activation(
                    sq[1][:, :], bq[:, 0:256], AF.Sin, scale=SEEDS[1]
                ).then_inc(s_feat, 1)
                nc.scalar.activation(
                    cq[1][:, :], bq[:, 0:256], AF.Sin,
                    bias=b_pi2[:, :], scale=SEEDS[1],
                ).then_inc(s_feat, 1)
                nc.scalar.activation(
                    sv[1][:, :], bv[:, :], AF.Sin, scale=SEEDS[1]
                ).then_inc(s_feat, 1)
                nc.scalar.activation(
                    cv[1][:, :], bv[:, :], AF.Sin,
                    bias=b_pi2[:, :], scale=SEEDS[1],
                ).then_inc(s_feat, 1)
                sqf = sqd = 0
                for (nf, nd), kind, a1, a2_, a3_ in ACT_MID:
                    if nf and sqf < nf:
                        act.wait_ge(s_feat, nf)
                        sqf = nf
                    if nd and sqd < nd:
                        act.wait_ge(s_dve, nd)
                        sqd = nd
                    if kind == "sq":
                        nc.scalar.activation(
                            a1[:, :], a2_[:, :], AF.Square
                        ).then_inc(s_sq, 1)
                    else:
                        nc.scalar.mul(
                            aq[a1][:, :], a2_[:, :], float(a3_)
                        ).then_inc(s_aact, 1)
                nc.scalar.copy(v16[:, :, 0:D], v_nat[:, :, 0:D]).then_inc(
                    s_v16, 1
                )
                for kt in range(KT):
                    act.wait_ge(s_mm, kt + 1)
                    nc.scalar.activation(
                        w_sb[:, kt, :], e_ps[kt][:, 0:256], AF.Exp
                    ).then_inc(s_w, 1)
                # Normalize on ACT (Copy with per-partition scale).
                for it in range(2):
                    act.wait_ge(s_rs, it + 1)
                    nc.scalar.mul(
                        o_sb[:, it, :], bo[it][:, 0:D], rs[it][:, :]
                    ).then_inc(s_norm, 1)
                nc.scalar.dma_start(
                    out=o_d[128:256, :], in_=o_sb[:, 1, :]
                ).then_inc(s_outd, 16)
                nc.scalar.dma_start(
                    out=o_d[128:256, :], in_=o_sb[:, 1, :]
                ).then_inc(s_outd, 16)



            @block.vector
            def _(dve):
                for i, op in enumerate(dve_ops):
                    for kind, val in dve_waits.get(i, ()):
                        dve.wait_ge(s_feat if kind == "feat" else s_sq, val)
                    if op[0] == "tt":
                        _, out, a, b, alu = op
                        nc.vector.tensor_tensor(
                            out[:, :], a[:, :], b[:, :], getattr(OP, alu)
                        ).then_inc(s_dve, 1)
                    elif op[0] == "stt":
                        _, out, a, scl, b, alu = op
                        nc.vector.scalar_tensor_tensor(
                            out[:, :], a[:, :], float(scl), b[:, :],
                            OP.mult, getattr(OP, alu),
                        ).then_inc(s_dve, 1)
                    elif op[0] == "stt":
                        _, out, a, scl, b, alu = op
                        nc.vector.scalar_tensor_tensor(
                            out[:, :], a[:, :], float(scl), b[:, :],
                            OP.mult, getattr(OP, alu),
                        ).then_inc(s_dve, 1)
                    elif op[0] == "ts":
                        _, out, a, s1, s2 = op
                        nc.vector.tensor_scalar(
                            out[:, :], a[:, :], s1, s2, OP.mult, OP.add
                        ).then_inc(s_dve, 1)
                    elif op[0] == "stt":
                        _, out, a, scl, b = op
                        nc.vector.scalar_tensor_tensor(
                            out[:, :], a[:, :], scl, b[:, :], OP.mult, OP.add
                        ).then_inc(s_dve, 1)
                    else:  # a2: inline alpha-scale
                        _, out, src, scl = op
                        nc.vector.tensor_scalar_mul(
                            out[:, :], src[:, :], float(scl)
                        ).then_inc(s_alpha2, 1)
                for it in range(2):
                    dve.wait_ge(s_o, it + 1)
                    nc.vector.reciprocal(
                        rs[it][:, :], bo[it][:, D : D + 1]
                    ).then_inc(s_rs, 1)

    return nc


def _get_nc():
    if "nc" not in _NC_CACHE:
        _NC_CACHE["nc"] = _build_nc()
    return _NC_CACHE["nc"]


def kernel_with_results(query, value, trace=False):
    import concourse.bass_utils as bass_utils

    query = np.ascontiguousarray(np.asarray(query, dtype=np.float32))
    value = np.ascontiguousarray(np.asarray(value, dtype=np.float32))
    assert query.shape == (B, TQ_FULL, D), query.shape
    assert value.shape == (B, TK, D), value.shape

    in_maps = []
    for c in range(N_CORES):
        b, half = c // 2, c % 2
        in_maps.append(
            {
                "query": np.ascontiguousarray(
                    query[b, half * TQ : (half + 1) * TQ, :]
                ),
                "value": np.ascontiguousarray(value[b]),
            }
        )

    res = bass_utils.run_bass_kernel_spmd(
        _get_nc(), in_maps, core_ids=list(range(N_CORES)), trace=trace
    )

    out = np.empty((B, TQ_FULL, D), dtype=np.float32)
    for c in range(N_CORES):
        b, half = c // 2, c % 2
        out[b, half * TQ : (half + 1) * TQ, :] = res.results[c]["out"]
    return out, res


def kernel(query, value):
    out, _ = kernel_with_results(query, value, trace=False)
    return out
